# revision 5
# baseline (speedup 1.0000x reference)
"""DeepSet (phi -> masked sum pool -> rho) Trainium2 Bass kernel.

Contract: kernel(**inputs) takes the FULL unsharded inputs of
nn_DeepSetArchitecture (inputs [512,2048,16] fp32 + phi/rho weights) and
returns the full [512] fp32 cost vector, distributing work over 8
NeuronCores internally.

Strategy
--------
Data-parallel over sets, but load-balanced by *valid point count*: each
set's valid prefix (rows before the first all-PAD row) is padded up to a
multiple of Q=256 points with zero-points and packed into a contiguous
per-core stream.  phi is applied to the packed stream with fp32r matmuls
(full PE rate at free-dim 512).  The masked sum pool is fused into the
L3 relu pass via the ACT engine's accum_out (per-256-column partial
sums -> "pieces").  A final PE transpose + matmul with a per-core 0/1
piece->set matrix (device data, so the SPMD program stays identical
across cores) turns pieces into per-set pooled vectors; a K=1 matmul
adds the -(pad_count)*phi(0) correction for the zero pad points.  rho
runs on-chip on the pooled [128, S_cap] tile.
"""

import numpy as np

PAD = -10000.0
B, N, F = 512, 2048, 16
NCORES = 8
Q = 256          # pooling piece quantum (pieces per set = plen/Q)
CH = 512         # matmul moving chunk (= 1 PSUM bank of fp32)
ST = 1024        # supertile columns (relu/e.w. pass granularity)
SLAB = 4096      # points per DMA slab ([128, 512] fp32, band-interleaved)

_CACHE = {}


# --------------------------------------------------------------------------
# host-side packing
# --------------------------------------------------------------------------

def _lengths_from_inputs(x):
    is_pad = np.all(x == PAD, axis=-1)                      # [B, N]
    any_pad = is_pad.any(axis=-1)
    first = np.where(any_pad, is_pad.argmax(axis=-1), N)
    return first.astype(np.int64)                           # valid prefix len


def _balance(plens):
    """Greedy assign sets to cores by padded length, biggest first."""
    order = np.argsort(-plens, kind="stable")
    loads = [0] * NCORES
    assign = [[] for _ in range(NCORES)]
    for b in order:
        c = int(np.argmin(loads))
        assign[c].append(int(b))
        loads[c] += int(plens[b])
    return assign, loads


def _phi0(params):
    """phi(0) in fp32 numpy (pad-point correction vector)."""
    h = np.zeros((1, F), np.float32)
    for i in range(4):
        w = params[f"phi_w{i}"]
        bi = params[f"phi_b{i}"]
        h = np.maximum(h @ w + bi, 0.0).astype(np.float32)
    return h[0]                                             # [128]


def pack_inputs(x, params):
    """Returns (shape_key, per_core_inmaps, unshard_info)."""
    x = np.asarray(x, np.float32)
    lengths = _lengths_from_inputs(x)
    plens = ((lengths + Q - 1) // Q) * Q                    # padded lengths
    assign, loads = _balance(plens)

    p_cap = ((max(loads) + SLAB - 1) // SLAB) * SLAB
    s_cap = max(len(a) for a in assign)
    n_pieces = p_cap // Q
    n_pg = (n_pieces + 127) // 128
    n_slab = p_cap // SLAB

    v0 = _phi0(params)

    in_maps = []
    for c in range(NCORES):
        pts = np.zeros((p_cap, F), np.float32)
        amat = np.zeros((n_pg * 128, s_cap), np.float32)
        negpad = np.zeros((1, s_cap), np.float32)
        col = 0
        for si, b in enumerate(assign[c]):
            L = int(lengths[b]); pl = int(plens[b])
            pts[col:col + L] = x[b, :L]
            amat[col // Q:(col + pl) // Q, si] = 1.0
            negpad[0, si] = -(pl - L)
            col += pl
        xt = np.ascontiguousarray(pts.T)                # [16, p_cap]
        m = {
            "xt": xt,
            "amat": amat.reshape(n_pg, 128, s_cap).copy(),
            "negpad": negpad,
            "vrow": v0.reshape(1, 128).copy(),
        }
        for i in range(4):
            m[f"w{i}"] = np.asarray(params[f"phi_w{i}"], np.float32)
            m[f"b{i}"] = np.asarray(params[f"phi_b{i}"], np.float32).reshape(-1, 1)
            m[f"rw{i}"] = np.asarray(params[f"rho_w{i}"], np.float32)
            m[f"rb{i}"] = np.asarray(params[f"rho_b{i}"], np.float32).reshape(-1, 1)
        in_maps.append(m)

    key = (p_cap, s_cap, n_pg)
    return key, in_maps, assign


# --------------------------------------------------------------------------
# device program (SPMD: one Bass module run on all 8 cores)
# --------------------------------------------------------------------------

def build_program(p_cap, s_cap, n_pg):
    import concourse.bass as bass
    import concourse.mybir as mybir
    import concourse.tile as tile
    from concourse import bacc
    from concourse.masks import make_identity

    f32 = mybir.dt.float32
    f32r = mybir.dt.float32r
    RELU = mybir.ActivationFunctionType.Relu
    IDENT = mybir.ActivationFunctionType.Identity
    ADD = mybir.AluOpType.add
    MAX = mybir.AluOpType.max

    n_slab = p_cap // SLAB
    n_st = p_cap // ST
    n_pieces = p_cap // Q

    nc = bacc.Bacc("TRN2", target_bir_lowering=False)

    xt_d = nc.dram_tensor("xt", [F, p_cap], f32r, kind="ExternalInput")
    amat_d = nc.dram_tensor("amat", [n_pg, 128, s_cap], f32, kind="ExternalInput")
    negpad_d = nc.dram_tensor("negpad", [1, s_cap], f32, kind="ExternalInput")
    vrow_d = nc.dram_tensor("vrow", [1, 128], f32, kind="ExternalInput")
    wd, bd, rwd, rbd = {}, {}, {}, {}
    for i in range(4):
        wshape = [F, 128] if i == 0 else [128, 128]
        wd[i] = nc.dram_tensor(f"w{i}", wshape, f32r, kind="ExternalInput")
        bd[i] = nc.dram_tensor(f"b{i}", [128, 1], f32, kind="ExternalInput")
        rwshape = [128, 1] if i == 3 else [128, 128]
        rbshape = [1, 1] if i == 3 else [128, 1]
        rwd[i] = nc.dram_tensor(f"rw{i}", rwshape, f32, kind="ExternalInput")
        rbd[i] = nc.dram_tensor(f"rb{i}", rbshape, f32, kind="ExternalInput")
    out_d = nc.dram_tensor("out", [1, s_cap], f32, kind="ExternalOutput")

    with tile.TileContext(nc) as tc:
        with (
            tc.tile_pool(name="const", bufs=1) as cp,
            tc.tile_pool(name="xp", bufs=8) as xp,
            tc.tile_pool(name="hp", bufs=2) as hp,
            tc.tile_pool(name="sp", bufs=2) as sp,
        ):
            # ---- resident constants ----
            def const(dram_ap, shape, tag, dt=f32):
                t = cp.tile(shape, dt, tag=tag)
                nc.sync.dma_start(t[:], dram_ap)
                return t

            w_sb = {i: const(wd[i][:], ([F, 128] if i == 0 else [128, 128]),
                             f"w{i}", f32r) for i in range(4)}
            b_sb = {i: const(bd[i][:], [128, 1], f"b{i}") for i in range(4)}
            rw_sb = {i: const(rwd[i][:], ([128, 1] if i == 3 else [128, 128]),
                              f"rw{i}") for i in range(4)}
            rb_sb = {i: const(rbd[i][:], ([1, 1] if i == 3 else [128, 1]),
                              f"rb{i}") for i in range(4)}
            a_sb = {g: const(amat_d[g], [128, s_cap], f"amat{g}")
                    for g in range(n_pg)}
            negpad_sb = const(negpad_d[:], [1, s_cap], "negpad")
            vrow_sb = const(vrow_d[:], [1, 128], "vrow")

            pieces = cp.tile([128, n_pg * 128], f32, tag="pieces")
            nc.vector.memset(pieces[:], 0.0)
            ident = cp.tile([128, 128], f32, tag="ident")
            make_identity(nc, ident[:])

            # ---- main phi loop ----
            psA_cm = tc.tile_pool(name="psA", bufs=3, space="PSUM")
            ps3_cm = tc.tile_pool(name="ps3", bufs=2, space="PSUM")
            psA, ps3 = psA_cm.__enter__(), ps3_cm.__enter__()
            for st in range(n_st):
                xt_t = xp.tile([F, ST], f32r, tag="xts")
                nc.sync.dma_start(xt_t[:], xt_d[:, st * ST:(st + 1) * ST])
                p0 = psA.tile([128, ST], f32, tag="pA")
                for j in range(ST // CH):
                    nc.tensor.matmul(p0[:, j * CH:(j + 1) * CH],
                                     w_sb[0][:],
                                     xt_t[:, j * CH:(j + 1) * CH])
                h0 = hp.tile([128, ST], f32r, tag="h0")
                nc.scalar.activation(h0[:], p0[:], RELU, bias=b_sb[0][:])

                p1 = psA.tile([128, ST], f32, tag="pA")
                for j in range(ST // CH):
                    nc.tensor.matmul(p1[:, j * CH:(j + 1) * CH], w_sb[1][:],
                                     h0[:, j * CH:(j + 1) * CH])
                h1 = hp.tile([128, ST], f32r, tag="h1")
                nc.vector.tensor_scalar(h1[:], p1[:], b_sb[1][:], 0.0, ADD, MAX)

                p2 = psA.tile([128, ST], f32, tag="pA")
                for j in range(ST // CH):
                    nc.tensor.matmul(p2[:, j * CH:(j + 1) * CH], w_sb[2][:],
                                     h1[:, j * CH:(j + 1) * CH])
                h2 = hp.tile([128, ST], f32r, tag="h2")
                nc.vector.tensor_scalar(h2[:], p2[:], b_sb[2][:], 0.0, ADD, MAX)

                for j in range(ST // CH):
                    p3 = ps3.tile([128, CH], f32, tag="p3")
                    nc.tensor.matmul(p3[:], w_sb[3][:],
                                     h2[:, j * CH:(j + 1) * CH])
                    for q in range(CH // Q):
                        p = st * (ST // Q) + j * (CH // Q) + q
                        scr = sp.tile([128, Q], f32, tag="scr")
                        nc.scalar.activation(
                            scr[:], p3[:, q * Q:(q + 1) * Q], RELU,
                            bias=b_sb[3][:],
                            accum_out=pieces[:, p:p + 1])

            ps3_cm.__exit__(None, None, None)
            psA_cm.__exit__(None, None, None)

            # ---- combine pieces -> pooled [128, s_cap] ----
            pst_cm = tc.tile_pool(name="pst", bufs=1, space="PSUM")
            pst = pst_cm.__enter__()
            pooled_p = pst.tile([128, s_cap], f32, tag="pooled")
            for g in range(n_pg):
                trp = pst.tile([128, 128], f32, tag="trp")
                nc.tensor.transpose(trp[:], pieces[:, g * 128:(g + 1) * 128],
                                    ident[:])
                ptg = sp.tile([128, 128], f32, tag="ptg")
                nc.vector.tensor_copy(ptg[:], trp[:])
                nc.tensor.matmul(pooled_p[:], ptg[:], a_sb[g][:],
                                 start=(g == 0), stop=False)
            nc.tensor.matmul(pooled_p[:], vrow_sb[:], negpad_sb[:],
                             start=False, stop=True)

            # ---- rho ----
            cur = sp.tile([128, s_cap], f32, tag="rho_h")
            nc.scalar.copy(cur[:], pooled_p[:])
            for i in range(3):
                pp = pst.tile([128, s_cap], f32, tag="rhop")
                nc.tensor.matmul(pp[:], rw_sb[i][:], cur[:])
                nxt = sp.tile([128, s_cap], f32, tag=f"rho_h{i}")
                nc.scalar.activation(nxt[:], pp[:], RELU, bias=rb_sb[i][:])
                cur = nxt
            po = pst.tile([1, s_cap], f32, tag="rhoo")
            nc.tensor.matmul(po[:], rw_sb[3][:], cur[:])
            out_sb = sp.tile([1, s_cap], f32, tag="outsb")
            nc.scalar.activation(out_sb[:], po[:], IDENT, bias=rb_sb[3][:])
            nc.sync.dma_start(out_d[:], out_sb[:])
            pst_cm.__exit__(None, None, None)

    nc.compile()
    return nc


# --------------------------------------------------------------------------
# entry point
# --------------------------------------------------------------------------

def kernel(**inputs):
    x = np.asarray(inputs["inputs"])
    params = {k: np.asarray(v) for k, v in inputs.items() if k != "inputs"}
    key, in_maps, assign = pack_inputs(x, params)

    if key not in _CACHE:
        _CACHE[key] = build_program(*key)
    nc = _CACHE[key]

    from concourse.bass_utils import run_bass_kernel_spmd
    res = run_bass_kernel_spmd(nc, in_maps, core_ids=list(range(NCORES)))

    cost = np.zeros((B,), np.float32)
    for c in range(NCORES):
        o = res.results[c]["out"][0]
        for si, b in enumerate(assign[c]):
            cost[b] = o[si]
    return cost
